# revision 37
# baseline (speedup 1.0000x reference)
"""Trainium2 Bass kernel for nn_Event_Critic_Net (dual-branch GAT critic).

Math: the reference reads the GAT output only at the LAST node of each
graph (graphs are 32 contiguous nodes), so only edges (n -> last(g))
contribute.  For those the softmax-weighted aggregation commutes with
the projection W:

    out_g = sigmoid( (sum_n alpha[n] x[n,:]) @ W + bias )
    alpha[n] = cnt[n] e^{z[n]} / (sum_n cnt[n] e^{z[n]} + 1e-16)
    z[n] = leaky_relu(x[n].w_src + x[last(g)].w_dst),  w_* = W @ att_*

Only ~7 of 32 nodes per graph have cnt>0, so the host compacts
contributors to K=16 slots per graph (8 graphs per 128-partition tile,
64 home tiles per core + overflow tiles for graphs with >16
contributors).  x is shipped once, node-major, pre-scaled by w_src so
a_src is a plain row-sum (DVE tensor_reduce); the projection uses
W' = W / w_src to undo the scaling.  Aggregation runs on the PE with
64-column stationary tiles (fast weight load) and the per-slot softmax
weights M as the 8-column moving operand.  Graphs are data-parallel
across 8 cores; each core sorts its 512 graphs by contributor count so
overflow slots land in accumulate-into-the-same-PSUM overflow tiles.
"""

import numpy as np
from contextlib import ExitStack

NC = 8
N = 131072
G = 4096
NPG = 32
S = 64
H = 128
GPC = G // NC          # 512 graphs per core
K = 16                 # slots per graph
TH = GPC * K // 128    # 64 home tiles per core
NEG = 0.2
NWARM = 36             # PE clock warm-up matmuls

_CACHE = {}


def _build_module(OVU, OVD):
    import concourse.tile as tile
    from concourse import bacc, mybir
    from concourse.alu_op_type import AluOpType as Alu

    f32 = mybir.dt.float32
    bf16 = mybir.dt.bfloat16
    Act = mybir.ActivationFunctionType
    AxX = mybir.AxisListType.X

    TU = TH + OVU
    TD = TH + OVD

    nc = bacc.Bacc("TRN2", target_bir_lowering=False, debug=False,
                   num_devices=NC)

    # ---- DRAM io ----
    FW = 68 + TU + TD            # cstF cols
    BW = 1536                    # cstB cols
    dram = {
        "u_xn": nc.dram_tensor("u_xn", [128, TU * S], bf16,
                               kind="ExternalInput"),
        "d_xn": nc.dram_tensor("d_xn", [128, TD * S], bf16,
                               kind="ExternalInput"),
        "cstF": nc.dram_tensor("cstF", [128, FW], f32,
                               kind="ExternalInput"),
        "cstB": nc.dram_tensor("cstB", [128, BW], bf16,
                               kind="ExternalInput"),
    }
    out_dram = nc.dram_tensor("out", [1, GPC], f32, kind="ExternalOutput")

    # chunk plan: [(t0, ntiles), ...] per branch
    def chunk_plan(T):
        n = 3
        base = T // n
        sizes = [base + (1 if i < T % n else 0) for i in range(n)]
        out = []
        t0 = 0
        for sz in sizes:
            out.append((t0, sz))
            t0 += sz
        return out

    CH = {"u": chunk_plan(TU), "d": chunk_plan(TD)}
    TT_ = {"u": TU, "d": TD}
    OV_ = {"u": OVU, "d": OVD}

    with tile.TileContext(nc) as tc, ExitStack() as ctx:
        const = ctx.enter_context(tc.tile_pool(name="const", bufs=1))
        xp = ctx.enter_context(tc.tile_pool(name="xp", bufs=1))
        wk = ctx.enter_context(tc.tile_pool(name="wk", bufs=1))
        pmix = ctx.enter_context(tc.tile_pool(name="pmix", bufs=2,
                                              space="PSUM"))
        pdn = ctx.enter_context(tc.tile_pool(name="pdn", bufs=2,
                                             space="PSUM"))
        py = ctx.enter_context(tc.tile_pool(name="py", bufs=2,
                                            space="PSUM"))
        pbig = ctx.enter_context(tc.tile_pool(name="pbig", bufs=2,
                                              space="PSUM"))

        # ---- phase 0: warm-up + constant loads ----
        wsrc = const.tile([64, 72], bf16, tag="wsrc")
        nc.vector.memset(wsrc[:], 1.0)
        zw = const.tile([128, 8], f32, tag="zw")
        nc.vector.memset(zw[:], 0.0)

        # constants go FIRST on each big queue (per-queue FIFO ensures
        # they land before the bulk x data)
        cstB = const.tile([128, BW], bf16, tag="cstB")
        nc.sync.dma_start(cstB[:], dram["cstB"].ap())
        cstF = const.tile([128, FW], f32, tag="cstF")
        nc.scalar.dma_start(cstF[:], dram["cstF"].ap())

        for wi in range(NWARM):
            w_ps = pmix.tile([128, 64], f32, tag="mix", name=f"warm{wi}")
            nc.tensor.matmul(w_ps[0:64, 0:8], wsrc[:, 0:64],
                             wsrc[:, 64:72], start=True, stop=True)
        zwe = const.tile([128, 8], f32, tag="zwe")
        nc.scalar.activation(zwe[:], zw[:], Act.Exp)

        # const views
        nbias = {"u": cstF[:, 0:1], "d": cstF[:, 1:2]}   # -(bias)
        eps = cstF[0:1, 2:3]
        ident64 = cstF[0:64, 4:68]
        CT = {"u": cstF[:, 68:68 + TU], "d": cstF[:, 68 + TU:68 + TU + TD]}
        Q16 = cstB[0:8, 0:128]
        B8 = cstB[:, 128:136]
        ones_col = cstB[:, 136:137]
        ones64 = cstB[0:1, 137:201]
        Wp = {"u": cstB[0:64, 201:329], "d": cstB[0:64, 329:457]}
        mlpW = cstB[:, 457:458]
        XL = {"u": cstB[0:64, 458:970], "d": cstB[0:64, 970:1482]}

        # ---- big input DMAs (chunked, interleaved across two queues) ----
        xt = {"u": [], "d": []}
        for p in ("u", "d"):
            for ci, (t0, nt) in enumerate(CH[p]):
                t = xp.tile([128, nt * S], bf16, tag=f"xn{p}{ci}",
                            name=f"xn{p}{ci}")
                xt[p].append(t)
        qmap = {("u", 0): nc.sync, ("u", 1): nc.sync, ("u", 2): nc.gpsimd,
                ("d", 0): nc.scalar, ("d", 1): nc.scalar,
                ("d", 2): nc.gpsimd}
        for ci in range(len(CH["u"])):
            for p in ("u", "d"):
                t0, nt = CH[p][ci]
                qmap[(p, ci)].dma_start(
                    xt[p][ci][:],
                    dram[f"{p}_xn"].ap()[:, t0 * S:(t0 + nt) * S])

        # ---- a_dst path (both branches) ----
        AD = {}
        for p in ("u", "d"):
            ad64 = wk.tile([64, 8], f32, tag=f"ad64{p}")
            nc.vector.tensor_reduce(
                ad64[:], XL[p].rearrange("p (j s) -> p j s", s=S),
                axis=AxX, op=Alu.add)
            tp = pmix.tile([128, 64], f32, tag="mix", name=f"tp{p}")
            tp = tp[0:8, :]
            nc.tensor.transpose(tp[:], ad64[:], ident64)
            adT = wk.tile([8, 64], bf16, tag=f"adT{p}")
            nc.scalar.activation(adT[:], tp[:], Act.Copy)
            ad_ps = pmix.tile([128, 64], f32, tag="mix", name=f"adps{p}")
            nc.tensor.matmul(ad_ps[:], Q16, adT[:], start=True, stop=True)
            a = wk.tile([128, TH], f32, tag=f"AD{p}")
            nc.scalar.activation(a[:], ad_ps[:], Act.Copy)
            AD[p] = a

        # ---- per-branch state ----
        st = {}
        for p in ("u", "d"):
            T = TT_[p]
            st[p] = {
                "AS": wk.tile([128, T], f32, tag=f"AS{p}", name=f"AS{p}"),
                "z": wk.tile([128, T], f32, tag=f"z{p}", name=f"z{p}"),
                "e": wk.tile([128, T], f32, tag=f"e{p}", name=f"e{p}"),
                "EX": wk.tile([128, T], f32, tag=f"EX{p}", name=f"EX{p}"),
                "P": wk.tile([128, T], f32, tag=f"P{p}", name=f"P{p}"),
                "M": wk.tile([128, T * 8], bf16, tag=f"M{p}",
                             name=f"M{p}"),
                "ynT": py.tile([64, GPC], f32, tag="ynT", name=f"ynT{p}"),
                "dn": pdn.tile([1, GPC], f32, tag="dn", name=f"dn{p}"),
                "ov": pmix.tile([128, 64], f32, tag="mix",
                                name=f"ov{p}"),
                "dnb": wk.tile([1, GPC], bf16, tag=f"dnb{p}",
                               name=f"dnb{p}"),
                "rbc": pbig.tile([64, GPC], f32, tag="big",
                                 name=f"rbc{p}"),
                "rinv": wk.tile([64, GPC], f32, tag=f"rinv{p}",
                                name=f"rinv{p}"),
                "ynrm": wk.tile([64, GPC], bf16, tag=f"ynrm{p}",
                                name=f"ynrm{p}"),
                "hT": pbig.tile([128, GPC], f32, tag="big",
                                name=f"hT{p}"),
                "exm": wk.tile([128, GPC], f32, tag=f"exm{p}",
                               name=f"exm{p}"),
                "ep1": wk.tile([128, GPC], f32, tag=f"ep1{p}",
                               name=f"ep1{p}"),
                "sg": wk.tile([128, GPC], f32, tag=f"sg{p}",
                              name=f"sg{p}"),
            }

        def reduce_chunk(p, ci):
            t0, nt = CH[p][ci]
            s = st[p]
            # stage 1 on gpsimd: add feature halves (bf16), halving the
            # vector-engine reduce volume
            h1 = wk.tile([128, nt * 32], bf16, tag=f"h1{p}{ci}",
                         name=f"h1{p}{ci}")
            x3 = xt[p][ci][:].rearrange("p (t s) -> p t s", s=S)
            nc.gpsimd.tensor_tensor(
                h1[:].rearrange("p (t s) -> p t s", s=32),
                x3[:, :, 0:32], x3[:, :, 32:64], op=Alu.add)
            nc.vector.tensor_reduce(
                s["AS"][:, t0:t0 + nt],
                h1[:].rearrange("p (t s) -> p t s", s=32),
                axis=AxX, op=Alu.add)

        def mchain_chunk(p, ci):
            t0, nt = CH[p][ci]
            s = st[p]
            OV = OV_[p]
            # z = AS + AD (home tiles; overflow tiles use AD block 0..)
            h0, h1 = t0, min(t0 + nt, TH)
            if h1 > h0:
                nc.gpsimd.tensor_tensor(
                    s["z"][:, h0:h1], s["AS"][:, h0:h1], AD[p][:, h0:h1],
                    op=Alu.add)
            if t0 + nt > TH:
                o0 = max(t0, TH)
                nb = t0 + nt - o0
                nc.gpsimd.tensor_tensor(
                    s["z"][:, o0:o0 + nb], s["AS"][:, o0:o0 + nb],
                    AD[p][:, 0:nb], op=Alu.add)
            sl = slice(t0, t0 + nt)
            nc.vector.scalar_tensor_tensor(
                s["e"][:, sl], s["z"][:, sl], NEG, s["z"][:, sl],
                op0=Alu.mult, op1=Alu.max)
            nc.scalar.activation(s["EX"][:, sl], s["e"][:, sl], Act.Exp)
            nc.gpsimd.tensor_tensor(
                s["P"][:, sl], s["EX"][:, sl], CT[p][:, sl], op=Alu.mult)

        def mbuild_chunk(p, ci):
            t0, nt = CH[p][ci]
            s = st[p]
            nc.vector.tensor_tensor(
                s["M"][:, 8 * t0:8 * (t0 + nt)]
                    .rearrange("p (t j) -> p t j", j=8),
                s["P"][:, t0:t0 + nt].rearrange("p (t o) -> p t o", o=1)
                    .to_broadcast((128, nt, 8)),
                B8.rearrange("p (o j) -> p o j", o=1)
                    .to_broadcast((128, nt, 8)),
                op=Alu.mult)

        def agg_chunk(p, ci):
            t0, nt = CH[p][ci]
            s = st[p]
            OV = OV_[p]
            x = xt[p][ci]
            for i in range(nt):
                tid = t0 + i
                if tid < TH:
                    nc.tensor.matmul(
                        s["ynT"][:, 8 * tid:8 * tid + 8],
                        x[:, S * i:S * (i + 1)],
                        s["M"][:, 8 * tid:8 * tid + 8],
                        start=True, stop=True)
                else:
                    b = tid - TH      # overflow level 1, own PSUM tile
                    nc.tensor.matmul(
                        s["ov"][0:64, 8 * b:8 * b + 8],
                        x[:, S * i:S * (i + 1)],
                        s["M"][:, 8 * tid:8 * tid + 8],
                        start=True, stop=True)
            # denominator for this chunk's home cols
            h0, h1 = t0, min(t0 + nt, TH)
            if h1 > h0:
                nc.tensor.matmul(
                    s["dn"][:, 8 * h0:8 * h1], ones_col,
                    s["M"][:, 8 * h0:8 * h1],
                    start=True, stop=True)
            if t0 + nt > TH:
                o0 = max(t0, TH)
                nb = t0 + nt - o0
                nc.tensor.matmul(
                    s["ov"][0:1, 8 * OV:8 * OV + 8 * nb], ones_col,
                    s["M"][:, 8 * o0:8 * (o0 + nb)],
                    start=True, stop=True)

        HS = GPC // 2    # column-stream split

        def tail_ov(p):
            s = st[p]
            OV = OV_[p]
            # fold overflow-tile partial sums into block 0
            ovsb = wk.tile([64, 16 * OV], f32, tag=f"ovsb{p}")
            nc.scalar.activation(ovsb[:], s["ov"][0:64, 0:16 * OV],
                                 Act.Copy)
            nc.vector.tensor_tensor(
                s["ynT"][:, 0:8 * OV], s["ynT"][:, 0:8 * OV],
                ovsb[:, 0:8 * OV], op=Alu.add)
            nc.vector.tensor_tensor(
                s["dn"][:, 0:8 * OV], s["dn"][:, 0:8 * OV],
                ovsb[0:1, 8 * OV:16 * OV], op=Alu.add)

        def tail_a(p, h):
            s = st[p]
            c = slice(h * HS, (h + 1) * HS)
            nc.scalar.activation(s["dnb"][:, c], s["dn"][:, c], Act.Copy,
                                 bias=1e-16)
            nc.tensor.matmul(s["rbc"][:, c], ones64, s["dnb"][:, c],
                             start=True, stop=True)
            nc.vector.reciprocal_approx_fast(s["rinv"][:, c],
                                             s["rbc"][:, c])
            nc.vector.tensor_tensor(s["ynrm"][:, c], s["ynT"][:, c],
                                    s["rinv"][:, c], op=Alu.mult)

        def tail_b(p, h):
            s = st[p]
            c = slice(h * HS, (h + 1) * HS)
            nc.tensor.matmul(s["hT"][:, c], Wp[p], s["ynrm"][:, c],
                             start=True, stop=True)
            nc.scalar.activation(s["exm"][:, c], s["hT"][:, c], Act.Exp,
                                 bias=nbias[p], scale=-1.0)
            nc.gpsimd.tensor_scalar(s["ep1"][:, c], s["exm"][:, c], 1.0,
                                    None, op0=Alu.add)
            nc.vector.reciprocal_approx_fast(s["sg"][:, c], s["ep1"][:, c])

        # ---- schedule ----
        for ci in range(len(CH["u"])):
            for p in ("u", "d"):
                reduce_chunk(p, ci)
                mchain_chunk(p, ci)
                mbuild_chunk(p, ci)
                agg_chunk(p, ci)
        for p in ("u", "d"):
            tail_ov(p)
            for h in (1, 0):     # stream 1 has no overflow dependency
                tail_a(p, h)
                tail_b(p, h)

        # ---- head ----
        prod = wk.tile([128, GPC], bf16, tag="prod")
        o_ps = pdn.tile([1, GPC], f32, tag="dn", name="o_ps")
        o_sb = wk.tile([1, GPC], f32, tag="o_sb")
        for h in (1, 0):
            c = slice(h * HS, (h + 1) * HS)
            nc.vector.tensor_tensor(prod[:, c], st["u"]["sg"][:, c],
                                    st["d"]["sg"][:, c], op=Alu.mult)
            nc.tensor.matmul(o_ps[:, c], mlpW, prod[:, c], start=True,
                             stop=True)
            nc.scalar.activation(o_sb[:, c], o_ps[:, c], Act.Copy)
        nc.sync.dma_start(out_dram.ap(), o_sb[:])

    nc.compile()
    return nc


def _get_module(OVU=1, OVD=1):
    key = ("nc", OVU, OVD)
    if key not in _CACHE:
        _CACHE[key] = _build_module(OVU, OVD)
    return _CACHE[key]


# ---------------- host-side prep ----------------

def _branch_struct(ei):
    src = np.asarray(ei[0]).astype(np.int64)
    dst = np.asarray(ei[1]).astype(np.int64)
    valid = (dst % NPG) == (NPG - 1)
    cnt = np.bincount(src[valid], minlength=N).astype(np.float32)
    contrib = (cnt > 0).reshape(G, NPG).sum(1)
    return cnt, contrib


def _clamp_w(w):
    w = np.asarray(w, np.float64).copy()
    tiny = np.abs(w) < 1e-4
    w[tiny] = np.where(w[tiny] < 0, -1e-4, 1e-4)
    return w


def _overflow_tiles(orders, cnt):
    """#level-1 overflow blocks needed (uniform across cores); supports
    counts up to 32 (level-1 only) which holds for this data."""
    nb = 0
    for order in orders:
        counts = np.array([(cnt[g * NPG:(g + 1) * NPG] > 0).sum()
                           for g in order])
        assert counts.max() <= 2 * K, "needs level-2 overflow support"
        ranks = np.nonzero(counts > K)[0]
        if len(ranks):
            nb = max(nb, int(ranks.max() // 8 + 1))
    return nb


def _pack_branch(x, cnt, orders, w_src, w_dst, OV):
    import ml_dtypes
    bf = ml_dtypes.bfloat16
    x = np.asarray(x, np.float32)
    wc = _clamp_w(w_src).astype(np.float32)
    T = TH + OV
    per_core = []
    for c in range(NC):
        order = orders[c]
        XN = np.zeros((128, T * S), np.float32)
        CT = np.zeros((128, T), np.float32)
        XL = np.zeros((64, 8 * S), np.float32)
        for r, g in enumerate(order):
            nodes = np.nonzero(cnt[g * NPG:(g + 1) * NPG] > 0)[0] + g * NPG
            t, j = r // 8, r % 8
            XL[t, j * S:(j + 1) * S] = x[(g + 1) * NPG - 1] * w_dst
            for l in (0, 1):
                seg = nodes[K * l:K * (l + 1)]
                if len(seg) == 0:
                    break
                tid = t if l == 0 else TH + t
                p0 = 16 * j
                XN[p0:p0 + len(seg), tid * S:tid * S + S] = x[seg] * wc
                CT[p0:p0 + len(seg), tid] = cnt[seg]
        per_core.append({"XN": XN.astype(bf), "CT": CT,
                         "XL": XL.astype(np.float32)})
    return per_core, wc


def _build_in_maps(inputs):
    import ml_dtypes
    bf = ml_dtypes.bfloat16

    cnt_u, con_u = _branch_struct(inputs["up_edge_index"])
    cnt_d, con_d = _branch_struct(inputs["down_edge_index"])
    orders = []
    for c in range(NC):
        g0 = c * GPC
        mx = np.maximum(con_u[g0:g0 + GPC], con_d[g0:g0 + GPC])
        orders.append(np.argsort(-mx, kind="stable") + g0)
    OVU = max(1, _overflow_tiles(orders, cnt_u))
    OVD = max(1, _overflow_tiles(orders, cnt_d))
    TU, TD = TH + OVU, TH + OVD

    pcs = {}
    shr = {}
    for pref, p, cnt, OV in (("up", "u", cnt_u, OVU),
                             ("down", "d", cnt_d, OVD)):
        W = np.asarray(inputs[f"{pref}_W"], np.float32)
        w_src = W @ np.asarray(inputs[f"{pref}_att_src"], np.float32)
        w_dst = W @ np.asarray(inputs[f"{pref}_att_dst"], np.float32)
        pcs[p], wc = _pack_branch(inputs[f"{pref}_x"], cnt, orders,
                                  w_src, w_dst, OV)
        shr[p] = {
            "Wp": (W / wc[:, None]).astype(np.float32),
            "nbias": -np.asarray(inputs[f"{pref}_bias"], np.float32),
        }

    FW = 68 + TU + TD
    cstF = np.zeros((128, FW), np.float32)
    cstF[:, 0] = shr["u"]["nbias"]
    cstF[:, 1] = shr["d"]["nbias"]
    cstF[0, 2] = 1e-16
    cstF[0:64, 4:68] = np.eye(64, dtype=np.float32)

    cstB = np.zeros((128, 1536), np.float32)
    pp = np.arange(128)
    Q16 = np.zeros((8, 128), np.float32)
    Q16[pp // 16, pp] = 1.0
    cstB[0:8, 0:128] = Q16
    B8 = np.zeros((128, 8), np.float32)
    B8[pp, pp // 16] = 1.0
    cstB[:, 128:136] = B8
    cstB[:, 136] = 1.0                      # ones_col
    cstB[0, 137:201] = 1.0                  # ones64 row
    cstB[0:64, 201:329] = shr["u"]["Wp"]
    cstB[0:64, 329:457] = shr["d"]["Wp"]
    cstB[:, 457] = np.asarray(inputs["mlp_W"], np.float32).reshape(H)

    in_maps = []
    for c in range(NC):
        m = {"cstB": None, "cstF": None}
        cf = cstF.copy()
        cf[:, 68:68 + TU] = pcs["u"][c]["CT"]
        cf[:, 68 + TU:68 + TU + TD] = pcs["d"][c]["CT"]
        cb = cstB.copy()
        cb[0:64, 458:970] = pcs["u"][c]["XL"]
        cb[0:64, 970:1482] = pcs["d"][c]["XL"]
        m["cstF"] = cf
        m["cstB"] = cb.astype(bf)
        m["u_xn"] = pcs["u"][c]["XN"]
        m["d_xn"] = pcs["d"][c]["XN"]
        in_maps.append(m)
    meta = {"orders": orders, "OVU": OVU, "OVD": OVD,
            "mlp_b": float(np.asarray(inputs["mlp_b"]).reshape(-1)[0])}
    return in_maps, meta


def assemble(results, meta):
    out = np.zeros((G, 1), np.float32)
    for c in range(NC):
        o = np.asarray(results[c]["out"], np.float32).reshape(GPC)
        out[meta["orders"][c], 0] = o + meta["mlp_b"]
    return out


def kernel(**inputs):
    from concourse.bass_utils import run_bass_kernel_spmd

    in_maps, meta = _build_in_maps(inputs)
    nc = _get_module(meta["OVU"], meta["OVD"])
    res = run_bass_kernel_spmd(nc, in_maps, core_ids=list(range(NC)))
    return assemble(res.results, meta)


# revision 43
# speedup vs baseline: 1.5862x; 1.5862x over previous
"""Trainium2 Bass kernel for nn_Event_Critic_Net (dual-branch GAT critic).

Math: the reference reads the GAT output only at the LAST node of each
graph (graphs are 32 contiguous nodes), so only edges (n -> last(g))
contribute.  For those the softmax-weighted aggregation commutes with
the projection W:

    out_g = sigmoid( (sum_n alpha[n] x[n,:]) @ W + bias )
    alpha[n] = cnt[n] e^{z[n]} / (sum_n cnt[n] e^{z[n]} + 1e-16)
    z[n] = leaky_relu(x[n].w_src + x[last(g)].w_dst),  w_* = W @ att_*

Only ~7 of 32 nodes per graph have cnt>0, so the host compacts
contributors to K=16 slots per graph (8 graphs per 128-partition tile,
64 home tiles per core + overflow tiles for graphs with >16
contributors).  x is shipped once, node-major, pre-scaled by w_src so
a_src is a plain row-sum (DVE tensor_reduce); the projection uses
W' = W / w_src to undo the scaling.  Aggregation runs on the PE with
64-column stationary tiles (fast weight load) and the per-slot softmax
weights M as the 8-column moving operand.  Graphs are data-parallel
across 8 cores; each core sorts its 512 graphs by contributor count so
overflow slots land in accumulate-into-the-same-PSUM overflow tiles.
"""

import numpy as np
from contextlib import ExitStack

NC = 8
N = 131072
G = 4096
NPG = 32
S = 64
H = 128
GPC = G // NC          # 512 graphs per core
K = 16                 # slots per graph
TH = GPC * K // 128    # 64 home tiles per core
NEG = 0.2
NWARM = 36             # PE clock warm-up matmuls

_CACHE = {}


def _build_module(OVU, OVD):
    import concourse.tile as tile
    from concourse import bacc, mybir
    from concourse.alu_op_type import AluOpType as Alu

    f32 = mybir.dt.float32
    bf16 = mybir.dt.bfloat16
    Act = mybir.ActivationFunctionType
    AxX = mybir.AxisListType.X

    TU = TH + OVU
    TD = TH + OVD

    nc = bacc.Bacc("TRN2", target_bir_lowering=False, debug=False,
                   num_devices=NC)

    # ---- DRAM io ----
    FW = 68 + TU + TD            # cstF cols
    BW = 1536                    # cstB cols
    dram = {
        "u_xn": nc.dram_tensor("u_xn", [128, TU * S], bf16,
                               kind="ExternalInput"),
        "d_xn": nc.dram_tensor("d_xn", [128, TD * S], bf16,
                               kind="ExternalInput"),
        "cstF": nc.dram_tensor("cstF", [128, FW], f32,
                               kind="ExternalInput"),
        "cstB": nc.dram_tensor("cstB", [128, BW], bf16,
                               kind="ExternalInput"),
    }
    out_dram = nc.dram_tensor("out", [1, GPC], f32, kind="ExternalOutput")

    # chunk plan: [(t0, ntiles), ...] per branch
    def chunk_plan(T):
        n = 3
        base = T // n
        sizes = [base + (1 if i < T % n else 0) for i in range(n)]
        out = []
        t0 = 0
        for sz in sizes:
            out.append((t0, sz))
            t0 += sz
        return out

    CH = {"u": chunk_plan(TU), "d": chunk_plan(TD)}
    TT_ = {"u": TU, "d": TD}
    OV_ = {"u": OVU, "d": OVD}

    with tile.TileContext(nc) as tc, ExitStack() as ctx:
        const = ctx.enter_context(tc.tile_pool(name="const", bufs=1))
        xp = ctx.enter_context(tc.tile_pool(name="xp", bufs=1))
        wk = ctx.enter_context(tc.tile_pool(name="wk", bufs=1))
        pmix = ctx.enter_context(tc.tile_pool(name="pmix", bufs=2,
                                              space="PSUM"))
        pdn = ctx.enter_context(tc.tile_pool(name="pdn", bufs=2,
                                             space="PSUM"))
        py = ctx.enter_context(tc.tile_pool(name="py", bufs=2,
                                            space="PSUM"))
        pbig = ctx.enter_context(tc.tile_pool(name="pbig", bufs=2,
                                              space="PSUM"))

        # ---- phase 0: warm-up + constant loads ----
        wsrc = const.tile([64, 72], bf16, tag="wsrc")
        nc.vector.memset(wsrc[:], 1.0)
        zw = const.tile([128, 8], f32, tag="zw")
        nc.vector.memset(zw[:], 0.0)

        # constants go FIRST on each big queue (per-queue FIFO ensures
        # they land before the bulk x data)
        cstB = const.tile([128, BW], bf16, tag="cstB")
        nc.sync.dma_start(cstB[:], dram["cstB"].ap())
        cstF = const.tile([128, FW], f32, tag="cstF")
        nc.scalar.dma_start(cstF[:], dram["cstF"].ap())

        for wi in range(NWARM):
            w_ps = pmix.tile([128, 64], f32, tag="mix", name=f"warm{wi}")
            nc.tensor.matmul(w_ps[0:64, 0:8], wsrc[:, 0:64],
                             wsrc[:, 64:72], start=True, stop=True)
        zwe = const.tile([128, 8], f32, tag="zwe")
        nc.scalar.activation(zwe[:], zw[:], Act.Exp)

        # const views
        pbias = {"u": cstF[:, 0:1], "d": cstF[:, 1:2]}   # bias
        eps = cstF[0:1, 2:3]
        ident64 = cstF[0:64, 4:68]
        CT = {"u": cstF[:, 68:68 + TU], "d": cstF[:, 68 + TU:68 + TU + TD]}
        Q16 = cstB[0:8, 0:128]
        B8 = cstB[:, 128:136]
        ones_col = cstB[:, 136:137]
        ones64 = cstB[0:1, 137:201]
        Wp = {"u": cstB[0:64, 201:329], "d": cstB[0:64, 329:457]}
        mlpW = cstB[:, 457:458]
        XL = {"u": cstB[0:64, 458:970], "d": cstB[0:64, 970:1482]}

        # ---- big input DMAs (chunked, interleaved across two queues) ----
        xt = {"u": [], "d": []}
        for p in ("u", "d"):
            for ci, (t0, nt) in enumerate(CH[p]):
                t = xp.tile([128, nt * S], bf16, tag=f"xn{p}{ci}",
                            name=f"xn{p}{ci}")
                xt[p].append(t)
        qmap = {("u", 0): nc.sync, ("u", 1): nc.sync, ("u", 2): nc.gpsimd,
                ("d", 0): nc.scalar, ("d", 1): nc.scalar,
                ("d", 2): nc.gpsimd}
        for ci in range(len(CH["u"])):
            for p in ("u", "d"):
                t0, nt = CH[p][ci]
                qmap[(p, ci)].dma_start(
                    xt[p][ci][:],
                    dram[f"{p}_xn"].ap()[:, t0 * S:(t0 + nt) * S])

        # ---- a_dst path (both branches) ----
        AD = {}
        for p in ("u", "d"):
            ad64 = wk.tile([64, 8], f32, tag=f"ad64{p}")
            nc.vector.tensor_reduce(
                ad64[:], XL[p].rearrange("p (j s) -> p j s", s=S),
                axis=AxX, op=Alu.add)
            tp = pmix.tile([128, 64], f32, tag="mix", name=f"tp{p}")
            tp = tp[0:8, :]
            nc.tensor.transpose(tp[:], ad64[:], ident64)
            adT = wk.tile([8, 64], bf16, tag=f"adT{p}")
            nc.scalar.activation(adT[:], tp[:], Act.Copy)
            ad_ps = pmix.tile([128, 64], f32, tag="mix", name=f"adps{p}")
            nc.tensor.matmul(ad_ps[:], Q16, adT[:], start=True, stop=True)
            a = wk.tile([128, TH], f32, tag=f"AD{p}")
            nc.scalar.activation(a[:], ad_ps[:], Act.Copy)
            AD[p] = a

        # ---- per-branch state ----
        st = {}
        for p in ("u", "d"):
            T = TT_[p]
            st[p] = {
                "AS": wk.tile([128, T], f32, tag=f"AS{p}", name=f"AS{p}"),
                "z": wk.tile([128, T], f32, tag=f"z{p}", name=f"z{p}"),
                "e": wk.tile([128, T], f32, tag=f"e{p}", name=f"e{p}"),
                "EX": wk.tile([128, T], f32, tag=f"EX{p}", name=f"EX{p}"),
                "P": wk.tile([128, T], f32, tag=f"P{p}", name=f"P{p}"),
                "M": wk.tile([128, T * 8], bf16, tag=f"M{p}",
                             name=f"M{p}"),
                "ynT": py.tile([64, GPC], f32, tag="ynT", name=f"ynT{p}"),
                "dn": pdn.tile([1, GPC], f32, tag="dn", name=f"dn{p}"),
                "ov": pmix.tile([128, 64], f32, tag="mix",
                                name=f"ov{p}"),
                "dnb": wk.tile([1, GPC], bf16, tag=f"dnb{p}",
                               name=f"dnb{p}"),
                "rbc": pbig.tile([64, GPC], f32, tag="big",
                                 name=f"rbc{p}"),
                "rinv": wk.tile([64, GPC], f32, tag=f"rinv{p}",
                                name=f"rinv{p}"),
                "ynrm": wk.tile([64, GPC], bf16, tag=f"ynrm{p}",
                                name=f"ynrm{p}"),
                "hT": pbig.tile([128, GPC], f32, tag="big",
                                name=f"hT{p}"),
                "sg": wk.tile([128, GPC], bf16, tag=f"sg{p}",
                              name=f"sg{p}"),
            }

        def reduce_chunk(p, ci):
            t0, nt = CH[p][ci]
            s = st[p]
            nc.vector.tensor_reduce(
                s["AS"][:, t0:t0 + nt],
                xt[p][ci][:].rearrange("p (t s) -> p t s", s=S),
                axis=AxX, op=Alu.add)

        def mchain_chunk(p, ci):
            t0, nt = CH[p][ci]
            s = st[p]
            OV = OV_[p]
            # z = AS + AD (home tiles; overflow tiles use AD block 0..)
            h0, h1 = t0, min(t0 + nt, TH)
            if h1 > h0:
                nc.gpsimd.tensor_tensor(
                    s["z"][:, h0:h1], s["AS"][:, h0:h1], AD[p][:, h0:h1],
                    op=Alu.add)
            if t0 + nt > TH:
                o0 = max(t0, TH)
                nb = t0 + nt - o0
                nc.gpsimd.tensor_tensor(
                    s["z"][:, o0:o0 + nb], s["AS"][:, o0:o0 + nb],
                    AD[p][:, 0:nb], op=Alu.add)
            sl = slice(t0, t0 + nt)
            nc.vector.scalar_tensor_tensor(
                s["e"][:, sl], s["z"][:, sl], NEG, s["z"][:, sl],
                op0=Alu.mult, op1=Alu.max)
            nc.scalar.activation(s["EX"][:, sl], s["e"][:, sl], Act.Exp)
            nc.gpsimd.tensor_tensor(
                s["P"][:, sl], s["EX"][:, sl], CT[p][:, sl], op=Alu.mult)

        def mbuild_chunk(p, ci):
            t0, nt = CH[p][ci]
            s = st[p]
            nc.vector.tensor_tensor(
                s["M"][:, 8 * t0:8 * (t0 + nt)]
                    .rearrange("p (t j) -> p t j", j=8),
                s["P"][:, t0:t0 + nt].rearrange("p (t o) -> p t o", o=1)
                    .to_broadcast((128, nt, 8)),
                B8.rearrange("p (o j) -> p o j", o=1)
                    .to_broadcast((128, nt, 8)),
                op=Alu.mult)

        def agg_chunk(p, ci):
            t0, nt = CH[p][ci]
            s = st[p]
            OV = OV_[p]
            x = xt[p][ci]
            for i in range(nt):
                tid = t0 + i
                if tid < TH:
                    nc.tensor.matmul(
                        s["ynT"][:, 8 * tid:8 * tid + 8],
                        x[:, S * i:S * (i + 1)],
                        s["M"][:, 8 * tid:8 * tid + 8],
                        start=True, stop=True)
                else:
                    b = tid - TH      # overflow level 1, own PSUM tile
                    nc.tensor.matmul(
                        s["ov"][0:64, 8 * b:8 * b + 8],
                        x[:, S * i:S * (i + 1)],
                        s["M"][:, 8 * tid:8 * tid + 8],
                        start=True, stop=True)
            # denominator for this chunk's home cols
            h0, h1 = t0, min(t0 + nt, TH)
            if h1 > h0:
                nc.tensor.matmul(
                    s["dn"][:, 8 * h0:8 * h1], ones_col,
                    s["M"][:, 8 * h0:8 * h1],
                    start=True, stop=True)
            if t0 + nt > TH:
                o0 = max(t0, TH)
                nb = t0 + nt - o0
                nc.tensor.matmul(
                    s["ov"][0:1, 8 * OV:8 * OV + 8 * nb], ones_col,
                    s["M"][:, 8 * o0:8 * (o0 + nb)],
                    start=True, stop=True)

        HS = GPC // 2    # column-stream split

        def tail_ov(p):
            s = st[p]
            OV = OV_[p]
            # fold overflow-tile partial sums into block 0
            ovsb = wk.tile([64, 16 * OV], f32, tag=f"ovsb{p}")
            nc.scalar.activation(ovsb[:], s["ov"][0:64, 0:16 * OV],
                                 Act.Copy)
            nc.vector.tensor_tensor(
                s["ynT"][:, 0:8 * OV], s["ynT"][:, 0:8 * OV],
                ovsb[:, 0:8 * OV], op=Alu.add)
            nc.vector.tensor_tensor(
                s["dn"][:, 0:8 * OV], s["dn"][:, 0:8 * OV],
                ovsb[0:1, 8 * OV:16 * OV], op=Alu.add)

        def tail_a(p, h):
            s = st[p]
            c = slice(h * HS, (h + 1) * HS)
            nc.scalar.activation(s["dnb"][:, c], s["dn"][:, c], Act.Copy,
                                 bias=1e-16)
            nc.tensor.matmul(s["rbc"][:, c], ones64, s["dnb"][:, c],
                             start=True, stop=True)
            nc.vector.reciprocal_approx_fast(s["rinv"][:, c],
                                             s["rbc"][:, c])
            nc.vector.tensor_tensor(s["ynrm"][:, c], s["ynT"][:, c],
                                    s["rinv"][:, c], op=Alu.mult)

        def tail_b(p, h):
            s = st[p]
            c = slice(h * HS, (h + 1) * HS)
            nc.tensor.matmul(s["hT"][:, c], Wp[p], s["ynrm"][:, c],
                             start=True, stop=True)
            nc.scalar.activation(s["sg"][:, c], s["hT"][:, c], Act.Sigmoid,
                                 bias=pbias[p])

        # ---- schedule ----
        for ci in range(len(CH["u"])):
            for p in ("u", "d"):
                reduce_chunk(p, ci)
                mchain_chunk(p, ci)
                mbuild_chunk(p, ci)
                agg_chunk(p, ci)
        for p in ("u", "d"):
            tail_ov(p)
            for h in (1, 0):     # stream 1 has no overflow dependency
                tail_a(p, h)
                tail_b(p, h)

        # ---- head ----
        prod = wk.tile([128, GPC], bf16, tag="prod")
        o_ps = pdn.tile([1, GPC], f32, tag="dn", name="o_ps")
        o_sb = wk.tile([1, GPC], f32, tag="o_sb")
        for h in (1, 0):
            c = slice(h * HS, (h + 1) * HS)
            nc.vector.tensor_tensor(prod[:, c], st["u"]["sg"][:, c],
                                    st["d"]["sg"][:, c], op=Alu.mult)
            nc.tensor.matmul(o_ps[:, c], mlpW, prod[:, c], start=True,
                             stop=True)
            nc.scalar.activation(o_sb[:, c], o_ps[:, c], Act.Copy)
        nc.sync.dma_start(out_dram.ap(), o_sb[:])

    nc.compile()
    return nc


def _get_module(OVU=1, OVD=1):
    key = ("nc", OVU, OVD)
    if key not in _CACHE:
        _CACHE[key] = _build_module(OVU, OVD)
    return _CACHE[key]


# ---------------- host-side prep ----------------

def _branch_struct(ei):
    src = np.asarray(ei[0]).astype(np.int64)
    dst = np.asarray(ei[1]).astype(np.int64)
    valid = (dst % NPG) == (NPG - 1)
    cnt = np.bincount(src[valid], minlength=N).astype(np.float32)
    contrib = (cnt > 0).reshape(G, NPG).sum(1)
    return cnt, contrib


def _clamp_w(w):
    w = np.asarray(w, np.float64).copy()
    tiny = np.abs(w) < 1e-4
    w[tiny] = np.where(w[tiny] < 0, -1e-4, 1e-4)
    return w


def _overflow_tiles(orders, cnt):
    """#level-1 overflow blocks needed (uniform across cores); supports
    counts up to 32 (level-1 only) which holds for this data."""
    nb = 0
    for order in orders:
        counts = np.array([(cnt[g * NPG:(g + 1) * NPG] > 0).sum()
                           for g in order])
        assert counts.max() <= 2 * K, "needs level-2 overflow support"
        ranks = np.nonzero(counts > K)[0]
        if len(ranks):
            nb = max(nb, int(ranks.max() // 8 + 1))
    return nb


def _pack_branch(x, cnt, orders, w_src, w_dst, OV):
    import ml_dtypes
    bf = ml_dtypes.bfloat16
    x = np.asarray(x, np.float32)
    wc = _clamp_w(w_src).astype(np.float32)
    T = TH + OV
    per_core = []
    for c in range(NC):
        order = orders[c]
        XN = np.zeros((128, T * S), np.float32)
        CT = np.zeros((128, T), np.float32)
        XL = np.zeros((64, 8 * S), np.float32)
        for r, g in enumerate(order):
            nodes = np.nonzero(cnt[g * NPG:(g + 1) * NPG] > 0)[0] + g * NPG
            t, j = r // 8, r % 8
            XL[t, j * S:(j + 1) * S] = x[(g + 1) * NPG - 1] * w_dst
            for l in (0, 1):
                seg = nodes[K * l:K * (l + 1)]
                if len(seg) == 0:
                    break
                tid = t if l == 0 else TH + t
                p0 = 16 * j
                XN[p0:p0 + len(seg), tid * S:tid * S + S] = x[seg] * wc
                CT[p0:p0 + len(seg), tid] = cnt[seg]
        per_core.append({"XN": XN.astype(bf), "CT": CT,
                         "XL": XL.astype(np.float32)})
    return per_core, wc


def _build_in_maps(inputs):
    import ml_dtypes
    bf = ml_dtypes.bfloat16

    cnt_u, con_u = _branch_struct(inputs["up_edge_index"])
    cnt_d, con_d = _branch_struct(inputs["down_edge_index"])
    orders = []
    for c in range(NC):
        g0 = c * GPC
        mx = np.maximum(con_u[g0:g0 + GPC], con_d[g0:g0 + GPC])
        orders.append(np.argsort(-mx, kind="stable") + g0)
    OVU = max(1, _overflow_tiles(orders, cnt_u))
    OVD = max(1, _overflow_tiles(orders, cnt_d))
    TU, TD = TH + OVU, TH + OVD

    pcs = {}
    shr = {}
    for pref, p, cnt, OV in (("up", "u", cnt_u, OVU),
                             ("down", "d", cnt_d, OVD)):
        W = np.asarray(inputs[f"{pref}_W"], np.float32)
        w_src = W @ np.asarray(inputs[f"{pref}_att_src"], np.float32)
        w_dst = W @ np.asarray(inputs[f"{pref}_att_dst"], np.float32)
        pcs[p], wc = _pack_branch(inputs[f"{pref}_x"], cnt, orders,
                                  w_src, w_dst, OV)
        shr[p] = {
            "Wp": (W / wc[:, None]).astype(np.float32),
            "bias": np.asarray(inputs[f"{pref}_bias"], np.float32),
        }

    FW = 68 + TU + TD
    cstF = np.zeros((128, FW), np.float32)
    cstF[:, 0] = shr["u"]["bias"]
    cstF[:, 1] = shr["d"]["bias"]
    cstF[0, 2] = 1e-16
    cstF[0:64, 4:68] = np.eye(64, dtype=np.float32)

    cstB = np.zeros((128, 1536), np.float32)
    pp = np.arange(128)
    Q16 = np.zeros((8, 128), np.float32)
    Q16[pp // 16, pp] = 1.0
    cstB[0:8, 0:128] = Q16
    B8 = np.zeros((128, 8), np.float32)
    B8[pp, pp // 16] = 1.0
    cstB[:, 128:136] = B8
    cstB[:, 136] = 1.0                      # ones_col
    cstB[0, 137:201] = 1.0                  # ones64 row
    cstB[0:64, 201:329] = shr["u"]["Wp"]
    cstB[0:64, 329:457] = shr["d"]["Wp"]
    cstB[:, 457] = np.asarray(inputs["mlp_W"], np.float32).reshape(H)

    in_maps = []
    for c in range(NC):
        m = {"cstB": None, "cstF": None}
        cf = cstF.copy()
        cf[:, 68:68 + TU] = pcs["u"][c]["CT"]
        cf[:, 68 + TU:68 + TU + TD] = pcs["d"][c]["CT"]
        cb = cstB.copy()
        cb[0:64, 458:970] = pcs["u"][c]["XL"]
        cb[0:64, 970:1482] = pcs["d"][c]["XL"]
        m["cstF"] = cf
        m["cstB"] = cb.astype(bf)
        m["u_xn"] = pcs["u"][c]["XN"]
        m["d_xn"] = pcs["d"][c]["XN"]
        in_maps.append(m)
    meta = {"orders": orders, "OVU": OVU, "OVD": OVD,
            "mlp_b": float(np.asarray(inputs["mlp_b"]).reshape(-1)[0])}
    return in_maps, meta


def assemble(results, meta):
    out = np.zeros((G, 1), np.float32)
    for c in range(NC):
        o = np.asarray(results[c]["out"], np.float32).reshape(GPC)
        out[meta["orders"][c], 0] = o + meta["mlp_b"]
    return out


def kernel(**inputs):
    from concourse.bass_utils import run_bass_kernel_spmd

    in_maps, meta = _build_in_maps(inputs)
    nc = _get_module(meta["OVU"], meta["OVD"])
    res = run_bass_kernel_spmd(nc, in_maps, core_ids=list(range(NC)))
    return assemble(res.results, meta)


# revision 44
# speedup vs baseline: 1.7817x; 1.1233x over previous
"""Trainium2 Bass kernel for nn_Event_Critic_Net (dual-branch GAT critic).

Math: the reference reads the GAT output only at the LAST node of each
graph (graphs are 32 contiguous nodes), so only edges (n -> last(g))
contribute.  For those the softmax-weighted aggregation commutes with
the projection W:

    out_g = sigmoid( (sum_n alpha[n] x[n,:]) @ W + bias )
    alpha[n] = cnt[n] e^{z[n]} / (sum_n cnt[n] e^{z[n]} + 1e-16)
    z[n] = leaky_relu(x[n].w_src + x[last(g)].w_dst),  w_* = W @ att_*

Only ~7 of 32 nodes per graph have cnt>0, so the host compacts
contributors to K=16 slots per graph (8 graphs per 128-partition tile,
64 home tiles per core + overflow tiles for graphs with >16
contributors).  x is shipped once, node-major, pre-scaled by w_src so
a_src is a plain row-sum (DVE tensor_reduce); the projection uses
W' = W / w_src to undo the scaling.  Aggregation runs on the PE with
64-column stationary tiles (fast weight load) and the per-slot softmax
weights M as the 8-column moving operand.  Graphs are data-parallel
across 8 cores; each core sorts its 512 graphs by contributor count so
overflow slots land in accumulate-into-the-same-PSUM overflow tiles.
"""

import numpy as np
from contextlib import ExitStack

NC = 8
N = 131072
G = 4096
NPG = 32
S = 64
H = 128
GPC = G // NC          # 512 graphs per core
K = 16                 # slots per graph
TH = GPC * K // 128    # 64 home tiles per core
NEG = 0.2
NWARM = 36             # PE clock warm-up matmuls

_CACHE = {}


def _build_module(OVU, OVD):
    import concourse.tile as tile
    from concourse import bacc, mybir
    from concourse.alu_op_type import AluOpType as Alu

    f32 = mybir.dt.float32
    bf16 = mybir.dt.bfloat16
    Act = mybir.ActivationFunctionType
    AxX = mybir.AxisListType.X

    TU = TH + OVU
    TD = TH + OVD

    nc = bacc.Bacc("TRN2", target_bir_lowering=False, debug=False,
                   num_devices=NC)

    # ---- DRAM io ----
    FW = 68 + TU + TD            # cstF cols
    BW = 1536                    # cstB cols
    dram = {
        "u_xn": nc.dram_tensor("u_xn", [128, TU * S], bf16,
                               kind="ExternalInput"),
        "d_xn": nc.dram_tensor("d_xn", [128, TD * S], bf16,
                               kind="ExternalInput"),
        "cstF": nc.dram_tensor("cstF", [128, FW], f32,
                               kind="ExternalInput"),
        "cstB": nc.dram_tensor("cstB", [128, BW], bf16,
                               kind="ExternalInput"),
    }
    out_dram = nc.dram_tensor("out", [1, GPC], f32, kind="ExternalOutput")

    # chunk plan: [(t0, ntiles), ...] per branch
    def chunk_plan(T):
        n = 3
        base = T // n
        sizes = [base + (1 if i < T % n else 0) for i in range(n)]
        out = []
        t0 = 0
        for sz in sizes:
            out.append((t0, sz))
            t0 += sz
        return out

    CH = {"u": chunk_plan(TU), "d": chunk_plan(TD)}
    TT_ = {"u": TU, "d": TD}
    OV_ = {"u": OVU, "d": OVD}

    with tile.TileContext(nc) as tc, ExitStack() as ctx:
        const = ctx.enter_context(tc.tile_pool(name="const", bufs=1))
        xp = ctx.enter_context(tc.tile_pool(name="xp", bufs=1))
        wk = ctx.enter_context(tc.tile_pool(name="wk", bufs=1))
        pmix = ctx.enter_context(tc.tile_pool(name="pmix", bufs=2,
                                              space="PSUM"))
        pdn = ctx.enter_context(tc.tile_pool(name="pdn", bufs=2,
                                             space="PSUM"))
        py = ctx.enter_context(tc.tile_pool(name="py", bufs=2,
                                            space="PSUM"))
        pbig = ctx.enter_context(tc.tile_pool(name="pbig", bufs=2,
                                              space="PSUM"))

        # ---- phase 0: warm-up + constant loads ----
        wsrc = const.tile([64, 72], bf16, tag="wsrc")
        nc.vector.memset(wsrc[:], 1.0)
        zw = const.tile([128, 8], f32, tag="zw")
        nc.vector.memset(zw[:], 0.0)

        # constants go FIRST on each big queue (per-queue FIFO ensures
        # they land before the bulk x data)
        cstB = const.tile([128, BW], bf16, tag="cstB")
        nc.sync.dma_start(cstB[:], dram["cstB"].ap())
        cstF = const.tile([128, FW], f32, tag="cstF")
        nc.scalar.dma_start(cstF[:], dram["cstF"].ap())

        for wi in range(NWARM):
            w_ps = pmix.tile([128, 64], f32, tag="mix", name=f"warm{wi}")
            nc.tensor.matmul(w_ps[0:64, 0:8], wsrc[:, 0:64],
                             wsrc[:, 64:72], start=True, stop=True)
        zwe = const.tile([128, 8], f32, tag="zwe")
        nc.scalar.activation(zwe[:], zw[:], Act.Exp)

        # const views
        pbias = {"u": cstF[:, 0:1], "d": cstF[:, 1:2]}   # bias
        eps = cstF[0:1, 2:3]
        ident64 = cstF[0:64, 4:68]
        CT = {"u": cstF[:, 68:68 + TU], "d": cstF[:, 68 + TU:68 + TU + TD]}
        Q16 = cstB[0:8, 0:128]
        B8 = cstB[:, 128:136]
        ones_col = cstB[:, 136:137]
        ones64 = cstB[0:1, 137:201]
        Wp = {"u": cstB[0:64, 201:329], "d": cstB[0:64, 329:457]}
        mlpW = cstB[:, 457:458]
        XL = {"u": cstB[0:64, 458:970], "d": cstB[0:64, 970:1482]}

        # ---- big input DMAs (chunked, interleaved across two queues) ----
        xt = {"u": [], "d": []}
        for p in ("u", "d"):
            for ci, (t0, nt) in enumerate(CH[p]):
                t = xp.tile([128, nt * S], bf16, tag=f"xn{p}{ci}",
                            name=f"xn{p}{ci}")
                xt[p].append(t)
        qmap = {("u", 0): nc.sync, ("u", 1): nc.sync, ("u", 2): nc.sync,
                ("d", 0): nc.scalar, ("d", 1): nc.scalar,
                ("d", 2): nc.scalar}
        for ci in range(len(CH["u"])):
            for p in ("u", "d"):
                t0, nt = CH[p][ci]
                qmap[(p, ci)].dma_start(
                    xt[p][ci][:],
                    dram[f"{p}_xn"].ap()[:, t0 * S:(t0 + nt) * S])

        # ---- a_dst path (both branches) ----
        AD = {}
        for p in ("u", "d"):
            ad64 = wk.tile([64, 8], f32, tag=f"ad64{p}")
            nc.vector.tensor_reduce(
                ad64[:], XL[p].rearrange("p (j s) -> p j s", s=S),
                axis=AxX, op=Alu.add)
            tp = pmix.tile([128, 64], f32, tag="mix", name=f"tp{p}")
            tp = tp[0:8, :]
            nc.tensor.transpose(tp[:], ad64[:], ident64)
            adT = wk.tile([8, 64], bf16, tag=f"adT{p}")
            nc.scalar.activation(adT[:], tp[:], Act.Copy)
            ad_ps = pmix.tile([128, 64], f32, tag="mix", name=f"adps{p}")
            nc.tensor.matmul(ad_ps[:], Q16, adT[:], start=True, stop=True)
            a = wk.tile([128, TH], f32, tag=f"AD{p}")
            nc.scalar.activation(a[:], ad_ps[:], Act.Copy)
            AD[p] = a

        # ---- per-branch state ----
        st = {}
        for p in ("u", "d"):
            T = TT_[p]
            st[p] = {
                "AS": wk.tile([128, T], f32, tag=f"AS{p}", name=f"AS{p}"),
                "z": wk.tile([128, T], f32, tag=f"z{p}", name=f"z{p}"),
                "e": wk.tile([128, T], f32, tag=f"e{p}", name=f"e{p}"),
                "EX": wk.tile([128, T], f32, tag=f"EX{p}", name=f"EX{p}"),
                "P": wk.tile([128, T], f32, tag=f"P{p}", name=f"P{p}"),
                "M": wk.tile([128, T * 8], bf16, tag=f"M{p}",
                             name=f"M{p}"),
                "ynT": py.tile([64, GPC], f32, tag="ynT", name=f"ynT{p}"),
                "dn": pdn.tile([1, GPC], f32, tag="dn", name=f"dn{p}"),
                "ov": pmix.tile([128, 64], f32, tag="mix",
                                name=f"ov{p}"),
                "dnb": wk.tile([1, GPC], bf16, tag=f"dnb{p}",
                               name=f"dnb{p}"),
                "rbc": pbig.tile([64, GPC], f32, tag="big",
                                 name=f"rbc{p}"),
                "rinv": wk.tile([64, GPC], f32, tag=f"rinv{p}",
                                name=f"rinv{p}"),
                "ynrm": wk.tile([64, GPC], bf16, tag=f"ynrm{p}",
                                name=f"ynrm{p}"),
                "hT": pbig.tile([128, GPC], f32, tag="big",
                                name=f"hT{p}"),
                "sg": wk.tile([128, GPC], bf16, tag=f"sg{p}",
                              name=f"sg{p}"),
            }

        def reduce_chunk(p, ci):
            t0, nt = CH[p][ci]
            s = st[p]
            nc.vector.tensor_reduce(
                s["AS"][:, t0:t0 + nt],
                xt[p][ci][:].rearrange("p (t s) -> p t s", s=S),
                axis=AxX, op=Alu.add)

        def mchain_chunk(p, ci):
            t0, nt = CH[p][ci]
            s = st[p]
            OV = OV_[p]
            # z = AS + AD (home tiles; overflow tiles use AD block 0..)
            h0, h1 = t0, min(t0 + nt, TH)
            if h1 > h0:
                nc.gpsimd.tensor_tensor(
                    s["z"][:, h0:h1], s["AS"][:, h0:h1], AD[p][:, h0:h1],
                    op=Alu.add)
            if t0 + nt > TH:
                o0 = max(t0, TH)
                nb = t0 + nt - o0
                nc.gpsimd.tensor_tensor(
                    s["z"][:, o0:o0 + nb], s["AS"][:, o0:o0 + nb],
                    AD[p][:, 0:nb], op=Alu.add)
            sl = slice(t0, t0 + nt)
            nc.vector.scalar_tensor_tensor(
                s["e"][:, sl], s["z"][:, sl], NEG, s["z"][:, sl],
                op0=Alu.mult, op1=Alu.max)
            nc.scalar.activation(s["EX"][:, sl], s["e"][:, sl], Act.Exp)
            nc.gpsimd.tensor_tensor(
                s["P"][:, sl], s["EX"][:, sl], CT[p][:, sl], op=Alu.mult)

        def mbuild_chunk(p, ci):
            t0, nt = CH[p][ci]
            s = st[p]
            nc.vector.tensor_tensor(
                s["M"][:, 8 * t0:8 * (t0 + nt)]
                    .rearrange("p (t j) -> p t j", j=8),
                s["P"][:, t0:t0 + nt].rearrange("p (t o) -> p t o", o=1)
                    .to_broadcast((128, nt, 8)),
                B8.rearrange("p (o j) -> p o j", o=1)
                    .to_broadcast((128, nt, 8)),
                op=Alu.mult)

        def agg_chunk(p, ci):
            t0, nt = CH[p][ci]
            s = st[p]
            OV = OV_[p]
            x = xt[p][ci]
            for i in range(nt):
                tid = t0 + i
                if tid < TH:
                    nc.tensor.matmul(
                        s["ynT"][:, 8 * tid:8 * tid + 8],
                        x[:, S * i:S * (i + 1)],
                        s["M"][:, 8 * tid:8 * tid + 8],
                        start=True, stop=True)
                else:
                    b = tid - TH      # overflow level 1, own PSUM tile
                    nc.tensor.matmul(
                        s["ov"][0:64, 8 * b:8 * b + 8],
                        x[:, S * i:S * (i + 1)],
                        s["M"][:, 8 * tid:8 * tid + 8],
                        start=True, stop=True)
            # denominator for this chunk's home cols
            h0, h1 = t0, min(t0 + nt, TH)
            if h1 > h0:
                nc.tensor.matmul(
                    s["dn"][:, 8 * h0:8 * h1], ones_col,
                    s["M"][:, 8 * h0:8 * h1],
                    start=True, stop=True)
            if t0 + nt > TH:
                o0 = max(t0, TH)
                nb = t0 + nt - o0
                nc.tensor.matmul(
                    s["ov"][0:1, 8 * OV:8 * OV + 8 * nb], ones_col,
                    s["M"][:, 8 * o0:8 * (o0 + nb)],
                    start=True, stop=True)

        HS = GPC // 2    # column-stream split

        def tail_ov(p):
            s = st[p]
            OV = OV_[p]
            # fold overflow-tile partial sums into block 0
            ovsb = wk.tile([64, 16 * OV], f32, tag=f"ovsb{p}")
            nc.scalar.activation(ovsb[:], s["ov"][0:64, 0:16 * OV],
                                 Act.Copy)
            nc.vector.tensor_tensor(
                s["ynT"][:, 0:8 * OV], s["ynT"][:, 0:8 * OV],
                ovsb[:, 0:8 * OV], op=Alu.add)
            nc.vector.tensor_tensor(
                s["dn"][:, 0:8 * OV], s["dn"][:, 0:8 * OV],
                ovsb[0:1, 8 * OV:16 * OV], op=Alu.add)

        def tail_a(p, h):
            s = st[p]
            c = slice(h * HS, (h + 1) * HS)
            nc.scalar.activation(s["dnb"][:, c], s["dn"][:, c], Act.Copy,
                                 bias=1e-16)
            nc.tensor.matmul(s["rbc"][:, c], ones64, s["dnb"][:, c],
                             start=True, stop=True)
            nc.vector.reciprocal_approx_fast(s["rinv"][:, c],
                                             s["rbc"][:, c])
            nc.vector.tensor_tensor(s["ynrm"][:, c], s["ynT"][:, c],
                                    s["rinv"][:, c], op=Alu.mult)

        def tail_b(p, h):
            s = st[p]
            c = slice(h * HS, (h + 1) * HS)
            nc.tensor.matmul(s["hT"][:, c], Wp[p], s["ynrm"][:, c],
                             start=True, stop=True)
            nc.scalar.activation(s["sg"][:, c], s["hT"][:, c], Act.Sigmoid,
                                 bias=pbias[p])

        # ---- schedule ----
        for ci in range(len(CH["u"])):
            for p in ("u", "d"):
                reduce_chunk(p, ci)
                mchain_chunk(p, ci)
                mbuild_chunk(p, ci)
                agg_chunk(p, ci)
        for p in ("u", "d"):
            tail_ov(p)
            for h in (1, 0):     # stream 1 has no overflow dependency
                tail_a(p, h)
                tail_b(p, h)

        # ---- head ----
        prod = wk.tile([128, GPC], bf16, tag="prod")
        o_ps = pdn.tile([1, GPC], f32, tag="dn", name="o_ps")
        o_sb = wk.tile([1, GPC], f32, tag="o_sb")
        for h in (1, 0):
            c = slice(h * HS, (h + 1) * HS)
            nc.vector.tensor_tensor(prod[:, c], st["u"]["sg"][:, c],
                                    st["d"]["sg"][:, c], op=Alu.mult)
            nc.tensor.matmul(o_ps[:, c], mlpW, prod[:, c], start=True,
                             stop=True)
            nc.scalar.activation(o_sb[:, c], o_ps[:, c], Act.Copy)
        nc.sync.dma_start(out_dram.ap(), o_sb[:])

    nc.compile()
    return nc


def _get_module(OVU=1, OVD=1):
    key = ("nc", OVU, OVD)
    if key not in _CACHE:
        _CACHE[key] = _build_module(OVU, OVD)
    return _CACHE[key]


# ---------------- host-side prep ----------------

def _branch_struct(ei):
    src = np.asarray(ei[0]).astype(np.int64)
    dst = np.asarray(ei[1]).astype(np.int64)
    valid = (dst % NPG) == (NPG - 1)
    cnt = np.bincount(src[valid], minlength=N).astype(np.float32)
    contrib = (cnt > 0).reshape(G, NPG).sum(1)
    return cnt, contrib


def _clamp_w(w):
    w = np.asarray(w, np.float64).copy()
    tiny = np.abs(w) < 1e-4
    w[tiny] = np.where(w[tiny] < 0, -1e-4, 1e-4)
    return w


def _overflow_tiles(orders, cnt):
    """#level-1 overflow blocks needed (uniform across cores); supports
    counts up to 32 (level-1 only) which holds for this data."""
    nb = 0
    for order in orders:
        counts = np.array([(cnt[g * NPG:(g + 1) * NPG] > 0).sum()
                           for g in order])
        assert counts.max() <= 2 * K, "needs level-2 overflow support"
        ranks = np.nonzero(counts > K)[0]
        if len(ranks):
            nb = max(nb, int(ranks.max() // 8 + 1))
    return nb


def _pack_branch(x, cnt, orders, w_src, w_dst, OV):
    import ml_dtypes
    bf = ml_dtypes.bfloat16
    x = np.asarray(x, np.float32)
    wc = _clamp_w(w_src).astype(np.float32)
    T = TH + OV
    per_core = []
    for c in range(NC):
        order = orders[c]
        XN = np.zeros((128, T * S), np.float32)
        CT = np.zeros((128, T), np.float32)
        XL = np.zeros((64, 8 * S), np.float32)
        for r, g in enumerate(order):
            nodes = np.nonzero(cnt[g * NPG:(g + 1) * NPG] > 0)[0] + g * NPG
            t, j = r // 8, r % 8
            XL[t, j * S:(j + 1) * S] = x[(g + 1) * NPG - 1] * w_dst
            for l in (0, 1):
                seg = nodes[K * l:K * (l + 1)]
                if len(seg) == 0:
                    break
                tid = t if l == 0 else TH + t
                p0 = 16 * j
                XN[p0:p0 + len(seg), tid * S:tid * S + S] = x[seg] * wc
                CT[p0:p0 + len(seg), tid] = cnt[seg]
        per_core.append({"XN": XN.astype(bf), "CT": CT,
                         "XL": XL.astype(np.float32)})
    return per_core, wc


def _build_in_maps(inputs):
    import ml_dtypes
    bf = ml_dtypes.bfloat16

    cnt_u, con_u = _branch_struct(inputs["up_edge_index"])
    cnt_d, con_d = _branch_struct(inputs["down_edge_index"])
    orders = []
    for c in range(NC):
        g0 = c * GPC
        mx = np.maximum(con_u[g0:g0 + GPC], con_d[g0:g0 + GPC])
        orders.append(np.argsort(-mx, kind="stable") + g0)
    OVU = max(1, _overflow_tiles(orders, cnt_u))
    OVD = max(1, _overflow_tiles(orders, cnt_d))
    TU, TD = TH + OVU, TH + OVD

    pcs = {}
    shr = {}
    for pref, p, cnt, OV in (("up", "u", cnt_u, OVU),
                             ("down", "d", cnt_d, OVD)):
        W = np.asarray(inputs[f"{pref}_W"], np.float32)
        w_src = W @ np.asarray(inputs[f"{pref}_att_src"], np.float32)
        w_dst = W @ np.asarray(inputs[f"{pref}_att_dst"], np.float32)
        pcs[p], wc = _pack_branch(inputs[f"{pref}_x"], cnt, orders,
                                  w_src, w_dst, OV)
        shr[p] = {
            "Wp": (W / wc[:, None]).astype(np.float32),
            "bias": np.asarray(inputs[f"{pref}_bias"], np.float32),
        }

    FW = 68 + TU + TD
    cstF = np.zeros((128, FW), np.float32)
    cstF[:, 0] = shr["u"]["bias"]
    cstF[:, 1] = shr["d"]["bias"]
    cstF[0, 2] = 1e-16
    cstF[0:64, 4:68] = np.eye(64, dtype=np.float32)

    cstB = np.zeros((128, 1536), np.float32)
    pp = np.arange(128)
    Q16 = np.zeros((8, 128), np.float32)
    Q16[pp // 16, pp] = 1.0
    cstB[0:8, 0:128] = Q16
    B8 = np.zeros((128, 8), np.float32)
    B8[pp, pp // 16] = 1.0
    cstB[:, 128:136] = B8
    cstB[:, 136] = 1.0                      # ones_col
    cstB[0, 137:201] = 1.0                  # ones64 row
    cstB[0:64, 201:329] = shr["u"]["Wp"]
    cstB[0:64, 329:457] = shr["d"]["Wp"]
    cstB[:, 457] = np.asarray(inputs["mlp_W"], np.float32).reshape(H)

    in_maps = []
    for c in range(NC):
        m = {"cstB": None, "cstF": None}
        cf = cstF.copy()
        cf[:, 68:68 + TU] = pcs["u"][c]["CT"]
        cf[:, 68 + TU:68 + TU + TD] = pcs["d"][c]["CT"]
        cb = cstB.copy()
        cb[0:64, 458:970] = pcs["u"][c]["XL"]
        cb[0:64, 970:1482] = pcs["d"][c]["XL"]
        m["cstF"] = cf
        m["cstB"] = cb.astype(bf)
        m["u_xn"] = pcs["u"][c]["XN"]
        m["d_xn"] = pcs["d"][c]["XN"]
        in_maps.append(m)
    meta = {"orders": orders, "OVU": OVU, "OVD": OVD,
            "mlp_b": float(np.asarray(inputs["mlp_b"]).reshape(-1)[0])}
    return in_maps, meta


def assemble(results, meta):
    out = np.zeros((G, 1), np.float32)
    for c in range(NC):
        o = np.asarray(results[c]["out"], np.float32).reshape(GPC)
        out[meta["orders"][c], 0] = o + meta["mlp_b"]
    return out


def kernel(**inputs):
    from concourse.bass_utils import run_bass_kernel_spmd

    in_maps, meta = _build_in_maps(inputs)
    nc = _get_module(meta["OVU"], meta["OVD"])
    res = run_bass_kernel_spmd(nc, in_maps, core_ids=list(range(NC)))
    return assemble(res.results, meta)
